# revision 1
# baseline (speedup 1.0000x reference)
"""Trainium2 Bass kernel for 2D DWT low-pass (db2): out = mh @ x @ mht per (b,c).

Shapes (hardcoded): input [8, 64, 512, 512] f32, matrix_h [256, 512],
matrix_h_t [512, 256], output [8, 64, 256, 256] f32.

Sharding: data-parallel over the batch dim - core b processes input[b]
(64 images of 512x512); the small filter matrix is replicated.

The filter matrices are banded 4-tap / stride-2 (mh[r, c] != 0 only for
c in [2r-1, 2r+2]; for H == W, matrix_h_t == matrix_h.T).

Best mode ('stencil12', ~242 us/round vs 368 us for the old stencil6
baseline; measured rel err ~2.3e-4, dominated by float32r matmul
rounding):
  stage 1 (H-direction): tmp[p, w] = sum_h mh[p, h] x[h, w] as PE matmuls
    with mht as the *stationary* operand, streamed as float32r (4x faster
    than plain fp32 on the PE for moving dim >= 256). For output p-tile m
    the band touches x row chunks 2m and 2m+1 plus one corner row; the
    corner contribution is folded into a third PSUM-accumulated matmul
    using a host-built one-hot weight tile (extra input "wcin"), so there
    are no per-image fixup DMAs. The h2 tap is folded into the PSUM->SBUF
    copy on the Scalar engine (activation Copy with scale).
  stage 2 (W-direction): out[q] = sum_t h_t tmp[2q-1+t] as a 3-op DVE
    stencil along the free dim with fused multiply-add on the h2-scaled
    tmp. No transposes anywhere.
  DMA: per-image 1 MB input loads on the SP HWDGE ring; output stores
    batched over 8 images (2 MB) on the ACT HWDGE ring so stores never
    head-of-line-block loads. This sits at ~92% of the pure-DMA roofline
    for this traffic (measured with a DMA-only probe kernel).

The stencil taps are extracted from the matrix_h_t actually passed in
(so a degenerate, e.g. all-zero, matrix still produces correct output).
"""

import os
import sys

sys.path.insert(0, "/opt/trn_rl_repo")

import numpy as np

import concourse.bass as bass
import concourse.tile as tile
from concourse import bacc, mybir
from concourse.bass_utils import run_bass_kernel_spmd

N_CORES = 8
C = 64          # images per core (channel dim; batch is the shard dim)
H = W = 512
P = 128         # SBUF partitions
KH = H // P     # 4 row chunks
NQ = 256        # output columns

F32 = mybir.dt.float32
F32R = mybir.dt.float32r
MULT = mybir.AluOpType.mult
ADD = mybir.AluOpType.add

MODE = os.environ.get("DWT_MODE", "stencil12")


def _window(k: int) -> tuple[int, int]:
    """Nonzero column range [lo, hi) of mht rows [128k, 128k+128)."""
    lo = max(0, 64 * k - 1)
    hi = min(NQ, 64 * k + 65)
    return lo, hi


def _even(ap):
    """[p, n] view of elements 0,2,4,... of a [p, 2n] AP."""
    n2 = ap.shape[-1]
    return ap.rearrange("p (w t) -> p w t", t=2)[:, :, 0]


def _odd(ap):
    n2 = ap.shape[-1]
    return ap.rearrange("p (w t) -> p w t", t=2)[:, :, 1]


def _emit_stencil3(nc, tc, x_d, mht_d, out_d, taps, rounds):
    """stencil with: h2 folded into the PSUM->SBUF copy (ACT activation
    scale), 3-op stage-2 stencil on DVE using tap ratios, corner fixups on
    the otherwise-idle GpSimd engine."""
    h0, h1, h2, h3 = (float(t) for t in taps)
    # stage-2 reads the h2-scaled tmp; ratios recover the other taps
    r0 = h0 / h2 if h2 else 0.0
    r3 = h3 / h2 if h2 else 0.0
    ib = int(os.environ.get("DWT_IB", "3"))
    pb = int(os.environ.get("DWT_PB", "4"))
    with (
        tc.tile_pool(name="consts", bufs=1) as cpool,
        tc.tile_pool(name="img", bufs=ib) as img_pool,
        tc.tile_pool(name="tmp", bufs=ib) as tmp_pool,
        tc.tile_pool(name="outp", bufs=ib) as out_pool,
        tc.tile_pool(name="ps1", bufs=pb, space=bass.MemorySpace.PSUM) as ps1_pool,
    ):
        mht_t = cpool.tile([P, KH, NQ], F32)
        nc.sync.dma_start(mht_t[:], mht_d.rearrange("(k p) q -> p k q", p=P))

        zb = [cpool.tile([P, W], F32, tag=f"zb{z}", name=f"zb{z}")
              for z in range(2)]
        for z in range(2):
            nc.gpsimd.memset(zb[z][:], 0.0)

        def one_image(i):
            img_t = img_pool.tile([P, KH, W], F32, tag="img")
            nc.sync.dma_start(img_t[:], x_d[i].rearrange("(k p) w -> p k w", p=P))
            zb_i = zb[i % 2]
            nc.sync.dma_start(zb_i[127:128, :], x_d[i, 256:257, :])
            xr = img_pool.tile([1, W], F32, tag="xr")
            nc.sync.dma_start(xr[:], x_d[i, 255:256, :])

            out_t = out_pool.tile([P, 2, NQ], F32, tag="out")
            for m in range(2):
                ps = ps1_pool.tile([P, W], F32, tag="ps1")
                for j in range(2):
                    k = 2 * m + j
                    lhsT = mht_t[:, k, m * P:(m + 1) * P]
                    nc.tensor.matmul(ps[:], lhsT, img_t[:, k, :],
                                     start=(j == 0), stop=(j == 1))
                # tmp_s = h2 * tmp  (scale folded into the ACT copy)
                tmp_m = tmp_pool.tile([P, W], F32, tag="tmp")
                nc.scalar.activation(
                    tmp_m[:], ps[:],
                    mybir.ActivationFunctionType.Copy, scale=h2)
                # corner fixups on GpSimd (values pre-scaled by h2):
                if m == 0:
                    # tmp_s[127] += h2*h3 * x[256]
                    nc.vector.scalar_tensor_tensor(
                        tmp_m[:], zb_i[:], h2 * h3, tmp_m[:], MULT, ADD)
                else:
                    # tmp_s[128] += h2*h0 * x[255]
                    nc.vector.scalar_tensor_tensor(
                        tmp_m[0:1, :], xr[:], h2 * h0, tmp_m[0:1, :], MULT, ADD)

                # stage 2 on scaled tmp: out[q] = (h1/h2)*ev_s[q] + od_s[q]
                #   + (h0/h2)*od_s[q-1] + (h3/h2)*ev_s[q+1], all over tmp_s
                acc = out_t[:, m, :]
                nc.vector.scalar_tensor_tensor(
                    acc, _even(tmp_m[:]), h1 / h2 if h2 else 0.0,
                    _odd(tmp_m[:]), MULT, ADD)
                nc.vector.scalar_tensor_tensor(
                    acc[:, 1:], _even(tmp_m[:, 1:511]), r0,
                    acc[:, 1:], MULT, ADD)
                nc.vector.scalar_tensor_tensor(
                    acc[:, 0:255], _even(tmp_m[:, 2:512]), r3,
                    acc[:, 0:255], MULT, ADD)
            nc.sync.dma_start(out_d[i].rearrange("(m p) q -> p m q", p=P), out_t[:])

        def body():
            for i in range(C):
                one_image(i)

        if rounds == 1:
            body()
        else:
            with tc.For_i(0, rounds, 1):
                body()


def _emit_stencil5(nc, tc, x_d, mht_d, out_d, taps, rounds):
    """stencil3, but the PSUM->SBUF copies deinterleave tmp (even/odd w
    halves via strided ACT reads, unit-stride writes) so every DVE stage-2
    and fixup operand is unit-stride (fp32 2x mode)."""
    h0, h1, h2, h3 = (float(t) for t in taps)
    r0 = h0 / h2 if h2 else 0.0
    r1 = h1 / h2 if h2 else 0.0
    r3 = h3 / h2 if h2 else 0.0
    ib = int(os.environ.get("DWT_IB", "3"))
    pb = int(os.environ.get("DWT_PB", "4"))
    with (
        tc.tile_pool(name="consts", bufs=1) as cpool,
        tc.tile_pool(name="img", bufs=ib) as img_pool,
        tc.tile_pool(name="tmp", bufs=ib) as tmp_pool,
        tc.tile_pool(name="outp", bufs=ib) as out_pool,
        tc.tile_pool(name="ps1", bufs=pb, space=bass.MemorySpace.PSUM) as ps1_pool,
    ):
        mht_t = cpool.tile([P, KH, NQ], F32)
        nc.sync.dma_start(mht_t[:], mht_d.rearrange("(k p) q -> p k q", p=P))

        zb = [cpool.tile([P, 2, NQ], F32, tag=f"zb{z}", name=f"zb{z}")
              for z in range(2)]
        for z in range(2):
            nc.gpsimd.memset(zb[z][:], 0.0)

        def one_image(i):
            img_t = img_pool.tile([P, KH, W], F32, tag="img")
            nc.sync.dma_start(img_t[:], x_d[i].rearrange("(k p) w -> p k w", p=P))
            zb_i = zb[i % 2]
            # deinterleaved corner rows (ev half then od half)
            nc.sync.dma_start(
                zb_i[127:128, :, :],
                x_d[i, 256:257, :].rearrange("r (w t) -> r t w", t=2))
            xr = img_pool.tile([1, 2, NQ], F32, tag="xr")
            nc.sync.dma_start(
                xr[:], x_d[i, 255:256, :].rearrange("r (w t) -> r t w", t=2))

            out_t = out_pool.tile([P, 2, NQ], F32, tag="out")
            for m in range(2):
                ps = ps1_pool.tile([P, W], F32, tag="ps1")
                for j in range(2):
                    k = 2 * m + j
                    lhsT = mht_t[:, k, m * P:(m + 1) * P]
                    nc.tensor.matmul(ps[:], lhsT, img_t[:, k, :],
                                     start=(j == 0), stop=(j == 1))
                # deinterleaving scaled copies: tmp_m[:,0,:] = h2*tmp[2q],
                # tmp_m[:,1,:] = h2*tmp[2q+1]
                tmp_m = tmp_pool.tile([P, 2, NQ], F32, tag="tmp")
                nc.scalar.activation(
                    tmp_m[:, 0, :], _even(ps[:]),
                    mybir.ActivationFunctionType.Copy, scale=h2)
                nc.scalar.activation(
                    tmp_m[:, 1, :], _odd(ps[:]),
                    mybir.ActivationFunctionType.Copy, scale=h2)
                if m == 0:
                    # tmp_s[127] += h2*h3 * x[256]
                    nc.vector.scalar_tensor_tensor(
                        tmp_m[:], zb_i[:], h2 * h3, tmp_m[:], MULT, ADD)
                else:
                    # tmp_s[128] += h2*h0 * x[255]
                    nc.vector.scalar_tensor_tensor(
                        tmp_m[0:1, :, :], xr[:], h2 * h0,
                        tmp_m[0:1, :, :], MULT, ADD)

                acc = out_t[:, m, :]
                nc.vector.scalar_tensor_tensor(
                    acc, tmp_m[:, 0, :], r1, tmp_m[:, 1, :], MULT, ADD)
                nc.vector.scalar_tensor_tensor(
                    acc[:, 1:], tmp_m[:, 1, 0:255], r0, acc[:, 1:], MULT, ADD)
                nc.vector.scalar_tensor_tensor(
                    acc[:, 0:255], tmp_m[:, 0, 1:256], r3,
                    acc[:, 0:255], MULT, ADD)
            nc.sync.dma_start(out_d[i].rearrange("(m p) q -> p m q", p=P), out_t[:])

        def body():
            for i in range(C):
                one_image(i)

        if rounds == 1:
            body()
        else:
            with tc.For_i(0, rounds, 1):
                body()


def _emit_stencil6(nc, tc, x_d, mht_d, out_d, taps, rounds):
    """stencil3 + corner fixups as GpSimd accumulate-DMAs: the corner rows
    x[:,255,:] / x[:,256,:] of all images are loaded deinterleaved once per
    round, pre-scaled in two batched DVE ops, and added into the tmp tiles
    with 2 KB SBUF->SBUF accum DMAs on the otherwise-idle software DGE."""
    h0, h1, h2, h3 = (float(t) for t in taps)
    r0 = h0 / h2 if h2 else 0.0
    r1 = h1 / h2 if h2 else 0.0
    r3 = h3 / h2 if h2 else 0.0
    ib = int(os.environ.get("DWT_IB", "3"))
    pb = int(os.environ.get("DWT_PB", "4"))
    with (
        tc.tile_pool(name="consts", bufs=1) as cpool,
        tc.tile_pool(name="img", bufs=ib) as img_pool,
        tc.tile_pool(name="tmp", bufs=ib) as tmp_pool,
        tc.tile_pool(name="outp", bufs=ib) as out_pool,
        tc.tile_pool(name="corner", bufs=2) as corner_pool,
        tc.tile_pool(name="ps1", bufs=pb, space=bass.MemorySpace.PSUM) as ps1_pool,
    ):
        mht_t = cpool.tile([P, KH, NQ], F32)
        nc.sync.dma_start(mht_t[:], mht_d.rearrange("(k p) q -> p k q", p=P))

        def body():
            # corner rows of all images: partition c = image c; [c, r, w]
            xc = corner_pool.tile([C, 2, W], F32, tag="xc")
            nc.sync.dma_start(xc[:], x_d[:, 255:257, :])
            sc = corner_pool.tile([C, 2, W], F32, tag="sc")
            # row 255 -> tmp[128] fixup (scale h2*h0); row 256 -> tmp[127]
            nc.vector.tensor_scalar_mul(sc[:, 0], xc[:, 0], h2 * h0)
            nc.vector.tensor_scalar_mul(sc[:, 1], xc[:, 1], h2 * h3)

            for i in range(C):
                img_t = img_pool.tile([P, KH, W], F32, tag="img")
                nc.sync.dma_start(img_t[:],
                                  x_d[i].rearrange("(k p) w -> p k w", p=P))

                out_t = out_pool.tile([P, 2, NQ], F32, tag="out")
                for m in range(2):
                    ps = ps1_pool.tile([P, W], F32, tag="ps1")
                    for j in range(2):
                        k = 2 * m + j
                        lhsT = mht_t[:, k, m * P:(m + 1) * P]
                        nc.tensor.matmul(ps[:], lhsT, img_t[:, k, :],
                                         start=(j == 0), stop=(j == 1))
                    tmp_m = tmp_pool.tile([P, W], F32, tag="tmp")
                    nc.scalar.activation(
                        tmp_m[:], ps[:],
                        mybir.ActivationFunctionType.Copy, scale=h2)
                    if m == 0:
                        # tmp_s[127] += h2*h3 * x[256]
                        nc.gpsimd.dma_start(tmp_m[127:128, :],
                                            sc[i:i + 1, 1, :],
                                            accum_op=ADD)
                    else:
                        # tmp_s[128] += h2*h0 * x[255]
                        nc.gpsimd.dma_start(tmp_m[0:1, :],
                                            sc[i:i + 1, 0, :],
                                            accum_op=ADD)

                    acc = out_t[:, m, :]
                    nc.vector.scalar_tensor_tensor(
                        acc, _even(tmp_m[:]), r1, _odd(tmp_m[:]), MULT, ADD)
                    nc.vector.scalar_tensor_tensor(
                        acc[:, 1:], _even(tmp_m[:, 1:511]), r0,
                        acc[:, 1:], MULT, ADD)
                    nc.vector.scalar_tensor_tensor(
                        acc[:, 0:255], _even(tmp_m[:, 2:512]), r3,
                        acc[:, 0:255], MULT, ADD)
                nc.sync.dma_start(out_d[i].rearrange("(m p) q -> p m q", p=P),
                                  out_t[:])

        if rounds == 1:
            body()
        else:
            with tc.For_i(0, rounds, 1):
                body()


def _emit_stencil9(nc, tc, x_d, mht_d, out_d, taps, rounds):
    """stencil6 with float32r matmuls: the PE streams fp32 data at 1
    cycle/row (vs 4 for plain fp32) when the moving dim is >= 256, cutting
    stage-1 PE time ~4x. Numerics checked on hardware against the fp32
    reference."""
    h0, h1, h2, h3 = (float(t) for t in taps)
    r0 = h0 / h2 if h2 else 0.0
    r1 = h1 / h2 if h2 else 0.0
    r3 = h3 / h2 if h2 else 0.0
    ib = int(os.environ.get("DWT_IB", "3"))
    pb = int(os.environ.get("DWT_PB", "4"))
    with (
        tc.tile_pool(name="consts", bufs=1) as cpool,
        tc.tile_pool(name="img", bufs=ib) as img_pool,
        tc.tile_pool(name="tmp", bufs=ib) as tmp_pool,
        tc.tile_pool(name="outp", bufs=ib) as out_pool,
        tc.tile_pool(name="corner", bufs=2) as corner_pool,
        tc.tile_pool(name="ps1", bufs=pb, space=bass.MemorySpace.PSUM) as ps1_pool,
    ):
        mht_t = cpool.tile([P, KH, NQ], F32R)
        nc.sync.dma_start(mht_t[:],
                          mht_d.rearrange("(k p) q -> p k q", p=P).bitcast(F32R))

        def body():
            xc = corner_pool.tile([C, 2, W], F32, tag="xc")
            nc.sync.dma_start(xc[:], x_d[:, 255:257, :])
            sc = corner_pool.tile([C, 2, W], F32, tag="sc")
            nc.vector.tensor_scalar_mul(sc[:, 0], xc[:, 0], h2 * h0)
            nc.vector.tensor_scalar_mul(sc[:, 1], xc[:, 1], h2 * h3)

            for i in range(C):
                img_t = img_pool.tile([P, KH, W], F32R, tag="img")
                nc.sync.dma_start(
                    img_t[:],
                    x_d[i].rearrange("(k p) w -> p k w", p=P).bitcast(F32R))

                out_t = out_pool.tile([P, 2, NQ], F32, tag="out")
                for m in range(2):
                    ps = ps1_pool.tile([P, W], F32, tag="ps1")
                    for j in range(2):
                        k = 2 * m + j
                        lhsT = mht_t[:, k, m * P:(m + 1) * P]
                        nc.tensor.matmul(ps[:], lhsT, img_t[:, k, :],
                                         start=(j == 0), stop=(j == 1))
                    tmp_m = tmp_pool.tile([P, W], F32, tag="tmp")
                    nc.scalar.activation(
                        tmp_m[:], ps[:],
                        mybir.ActivationFunctionType.Copy, scale=h2)
                    if m == 0:
                        # tmp_s[127] += h2*h3 * x[256]
                        nc.gpsimd.dma_start(tmp_m[127:128, :],
                                            sc[i:i + 1, 1, :],
                                            accum_op=ADD)
                    else:
                        # tmp_s[128] += h2*h0 * x[255]
                        nc.gpsimd.dma_start(tmp_m[0:1, :],
                                            sc[i:i + 1, 0, :],
                                            accum_op=ADD)

                    acc = out_t[:, m, :]
                    nc.vector.scalar_tensor_tensor(
                        acc, _even(tmp_m[:]), r1, _odd(tmp_m[:]), MULT, ADD)
                    nc.vector.scalar_tensor_tensor(
                        acc[:, 1:], _even(tmp_m[:, 1:511]), r0,
                        acc[:, 1:], MULT, ADD)
                    nc.vector.scalar_tensor_tensor(
                        acc[:, 0:255], _even(tmp_m[:, 2:512]), r3,
                        acc[:, 0:255], MULT, ADD)
                nc.sync.dma_start(out_d[i].rearrange("(m p) q -> p m q", p=P),
                                  out_t[:])

        if rounds == 1:
            body()
        else:
            with tc.For_i(0, rounds, 1):
                body()


def _emit_stencil10(nc, tc, x_d, mht_d, out_d, taps, rounds, out_dt=F32):
    """stencil9 (f32r matmuls) + input loads batched over LB images and
    output stores batched over GB images, stores issued via nc.scalar (ACT
    HWDGE ring) so they never head-of-line-block the input loads on the SP
    HWDGE ring. out_dt=BF16 halves store traffic (mode stencil11)."""
    h0, h1, h2, h3 = (float(t) for t in taps)
    r0 = h0 / h2 if h2 else 0.0
    r1 = h1 / h2 if h2 else 0.0
    r3 = h3 / h2 if h2 else 0.0
    ib = int(os.environ.get("DWT_IB", "4"))
    pb = int(os.environ.get("DWT_PB", "4"))
    GB = int(os.environ.get("DWT_GB", "4"))
    LB = int(os.environ.get("DWT_LB", "1"))
    mm_dt = F32R if os.environ.get("DWT_MM", "f32") == "f32r" else F32
    with (
        tc.tile_pool(name="consts", bufs=1) as cpool,
        tc.tile_pool(name="img", bufs=ib) as img_pool,
        tc.tile_pool(name="tmp", bufs=ib) as tmp_pool,
        tc.tile_pool(name="outp", bufs=2) as out_pool,
        tc.tile_pool(name="corner", bufs=2) as corner_pool,
        tc.tile_pool(name="ps1", bufs=pb, space=bass.MemorySpace.PSUM) as ps1_pool,
    ):
        mht_t = cpool.tile([P, KH, NQ], mm_dt)
        nc.sync.dma_start(mht_t[:],
                          mht_d.rearrange("(k p) q -> p k q", p=P).bitcast(mm_dt))

        def body():
            xc = corner_pool.tile([C, 2, W], F32, tag="xc")
            nc.sync.dma_start(xc[:], x_d[:, 255:257, :])
            sc = corner_pool.tile([C, 2, W], F32, tag="sc")
            nc.vector.tensor_scalar_mul(sc[:, 0], xc[:, 0], h2 * h0)
            nc.vector.tensor_scalar_mul(sc[:, 1], xc[:, 1], h2 * h3)

            imgs = {}

            def load(i0):
                t = img_pool.tile([P, LB, KH, W], mm_dt, tag="img")
                nc.sync.dma_start(
                    t[:],
                    x_d[i0:i0 + LB]
                    .rearrange("c (k p) w -> p c k w", p=P).bitcast(mm_dt))
                imgs[i0] = t

            load(0)
            for i0 in range(0, C, GB):
                out_t = out_pool.tile([P, GB, 2, NQ], out_dt, tag="out")
                for ci in range(GB):
                    i = i0 + ci
                    if i % LB == 0:
                        if i + LB < C:
                            load(i + LB)
                        img_t = imgs.pop(i)
                    for m in range(2):
                        ps = ps1_pool.tile([P, W], F32, tag="ps1")
                        for j in range(2):
                            k = 2 * m + j
                            lhsT = mht_t[:, k, m * P:(m + 1) * P]
                            nc.tensor.matmul(ps[:], lhsT,
                                             img_t[:, i % LB, k, :],
                                             start=(j == 0), stop=(j == 1))
                        tmp_m = tmp_pool.tile([P, W], F32, tag="tmp")
                        nc.scalar.activation(
                            tmp_m[:], ps[:],
                            mybir.ActivationFunctionType.Copy, scale=h2)
                        if m == 0:
                            nc.gpsimd.dma_start(tmp_m[127:128, :],
                                                sc[i:i + 1, 1, :],
                                                accum_op=ADD)
                        else:
                            nc.gpsimd.dma_start(tmp_m[0:1, :],
                                                sc[i:i + 1, 0, :],
                                                accum_op=ADD)

                        acc = out_t[:, ci, m, :]
                        nc.vector.scalar_tensor_tensor(
                            acc, _even(tmp_m[:]), r1, _odd(tmp_m[:]),
                            MULT, ADD)
                        nc.vector.scalar_tensor_tensor(
                            acc[:, 1:], _even(tmp_m[:, 1:511]), r0,
                            acc[:, 1:], MULT, ADD)
                        nc.vector.scalar_tensor_tensor(
                            acc[:, 0:255], _even(tmp_m[:, 2:512]), r3,
                            acc[:, 0:255], MULT, ADD)
                nc.scalar.dma_start(
                    out_d[i0:i0 + GB].rearrange("c (m p) q -> p (c m) q", p=P),
                    out_t[:].rearrange("p c m q -> p (c m) q"))

        if rounds == 1:
            body()
        else:
            with tc.For_i(0, rounds, 1):
                body()


def _emit_stencil12(nc, tc, x_d, mht_d, wc_d, out_d, taps, rounds, out_dt=F32):
    """stencil10, but the two band-corner rows are folded into a third
    PSUM-accumulated matmul per m-tile (one-hot corner weight tiles passed
    in as a small extra host-built input), eliminating the per-image SWDGE
    accum-DMA fixups and the per-round corner-row preprocessing."""
    h0, h1, h2, h3 = (float(t) for t in taps)
    r0 = h0 / h2 if h2 else 0.0
    r1 = h1 / h2 if h2 else 0.0
    r3 = h3 / h2 if h2 else 0.0
    ib = int(os.environ.get("DWT_IB", "4"))
    pb = int(os.environ.get("DWT_PB", "4"))
    GB = int(os.environ.get("DWT_GB", "8"))
    LB = int(os.environ.get("DWT_LB", "1"))
    ALT = os.environ.get("DWT_ALT", "0") == "1"
    mm_dt = F32R if os.environ.get("DWT_MM", "f32r") == "f32r" else F32
    with (
        tc.tile_pool(name="consts", bufs=1) as cpool,
        tc.tile_pool(name="img", bufs=ib) as img_pool,
        tc.tile_pool(name="tmp", bufs=ib) as tmp_pool,
        tc.tile_pool(name="outp", bufs=2) as out_pool,
        tc.tile_pool(name="ps1", bufs=pb, space=bass.MemorySpace.PSUM) as ps1_pool,
    ):
        mht_t = cpool.tile([P, KH, NQ], mm_dt)
        nc.sync.dma_start(mht_t[:],
                          mht_d.rearrange("(k p) q -> p k q", p=P).bitcast(mm_dt))
        # corner weight tiles (host-built): wc[:, 0, :] adds h3*x[256] to
        # tmp row 127 (rhs chunk k=2); wc[:, 1, :] adds h0*x[255] to tmp
        # row 128 (rhs chunk k=1).
        wc = cpool.tile([P, 2, P], mm_dt, name="wc")
        nc.sync.dma_start(wc[:], wc_d.bitcast(mm_dt))

        def body():
            imgs = {}

            def load(i0):
                t = img_pool.tile([P, LB, KH, W], mm_dt, tag="img")
                eng = nc.scalar if (ALT and (i0 // LB) % 2) else nc.sync
                eng.dma_start(
                    t[:],
                    x_d[i0:i0 + LB]
                    .rearrange("c (k p) w -> p c k w", p=P).bitcast(mm_dt))
                imgs[i0] = t

            load(0)
            img_t = None
            for i0 in range(0, C, GB):
                out_t = out_pool.tile([P, GB, 2, NQ], out_dt, tag="out")
                for ci in range(GB):
                    i = i0 + ci
                    if i % LB == 0:
                        if i + LB < C:
                            load(i + LB)
                        img_t = imgs.pop(i)
                    for m in range(2):
                        ps = ps1_pool.tile([P, W], F32, tag="ps1")
                        for j in range(2):
                            k = 2 * m + j
                            lhsT = mht_t[:, k, m * P:(m + 1) * P]
                            nc.tensor.matmul(ps[:], lhsT,
                                             img_t[:, i % LB, k, :],
                                             start=(j == 0), stop=False)
                        kc = 2 if m == 0 else 1
                        nc.tensor.matmul(ps[:], wc[:, m, :],
                                         img_t[:, i % LB, kc, :],
                                         start=False, stop=True)
                        tmp_m = tmp_pool.tile([P, W], F32, tag="tmp")
                        nc.scalar.activation(
                            tmp_m[:], ps[:],
                            mybir.ActivationFunctionType.Copy, scale=h2)

                        acc = out_t[:, ci, m, :]
                        nc.vector.scalar_tensor_tensor(
                            acc, _even(tmp_m[:]), r1, _odd(tmp_m[:]),
                            MULT, ADD)
                        nc.vector.scalar_tensor_tensor(
                            acc[:, 1:], _even(tmp_m[:, 1:511]), r0,
                            acc[:, 1:], MULT, ADD)
                        nc.vector.scalar_tensor_tensor(
                            acc[:, 0:255], _even(tmp_m[:, 2:512]), r3,
                            acc[:, 0:255], MULT, ADD)
                seng = nc.gpsimd if ALT else nc.scalar
                seng.dma_start(
                    out_d[i0:i0 + GB].rearrange("c (m p) q -> p (c m) q", p=P),
                    out_t[:].rearrange("p c m q -> p (c m) q"))

        if rounds == 1:
            body()
        else:
            with tc.For_i(0, rounds, 1):
                body()


def _emit_stencil14(nc, tc, x_d, mht_d, wc_d, out_d, taps, rounds, out_dt=F32):
    """stencil12 with the row->partition mapping flipped to '(p k)':
    partition p holds x rows 4p..4p+3, so each partition's load source is
    8 KB contiguous (128 descriptors of 8 KB per 1 MB image instead of 512
    of 2 KB) for ~20% better HBM read efficiency. Stage-1 matmuls contract
    over 64-partition halves (K=64, 4 slots + 1 corner matmul per tile)."""
    h0, h1, h2, h3 = (float(t) for t in taps)
    r0 = h0 / h2 if h2 else 0.0
    r1 = h1 / h2 if h2 else 0.0
    r3 = h3 / h2 if h2 else 0.0
    ib = int(os.environ.get("DWT_IB", "4"))
    pb = int(os.environ.get("DWT_PB", "4"))
    GB = int(os.environ.get("DWT_GB", "8"))
    mm_dt = F32R if os.environ.get("DWT_MM", "f32r") == "f32r" else F32
    with (
        tc.tile_pool(name="consts", bufs=1) as cpool,
        tc.tile_pool(name="img", bufs=ib) as img_pool,
        tc.tile_pool(name="tmp", bufs=ib) as tmp_pool,
        tc.tile_pool(name="outp", bufs=2) as out_pool,
        tc.tile_pool(name="ps1", bufs=pb, space=bass.MemorySpace.PSUM) as ps1_pool,
    ):
        mht_t = cpool.tile([P, KH, NQ], mm_dt)
        nc.sync.dma_start(mht_t[:],
                          mht_d.rearrange("(p k) q -> p k q", k=KH)
                          .bitcast(mm_dt))
        # host-built one-hot corner weights ('(p k)' layout):
        # wc[64, 0, 127] = h3 (x row 256 -> tmp row 127, rhs slot 0,
        # partitions 64..127); wc[63, 1, 0] = h0 (x row 255 -> tmp row
        # 128, rhs slot 3, partitions 0..63).
        wc = cpool.tile([P, 2, P], mm_dt, name="wc")
        nc.sync.dma_start(wc[:], wc_d.bitcast(mm_dt))

        def body():
            for i0 in range(0, C, GB):
                out_t = out_pool.tile([P, GB, 2, NQ], out_dt, tag="out")
                for ci in range(GB):
                    i = i0 + ci
                    img_t = img_pool.tile([P, KH, W], mm_dt, tag="img")
                    nc.sync.dma_start(
                        img_t[:],
                        x_d[i].rearrange("(p k) w -> p k w", k=KH)
                        .bitcast(mm_dt))
                    for m in range(2):
                        pbase = 64 * m
                        ps = ps1_pool.tile([P, W], F32, tag="ps1")
                        for k in range(KH):
                            lhsT = mht_t[pbase:pbase + 64, k,
                                         m * P:(m + 1) * P]
                            nc.tensor.matmul(ps[:], lhsT,
                                             img_t[pbase:pbase + 64, k, :],
                                             start=(k == 0), stop=False)
                        cb = 64 - pbase  # corner partitions: other half
                        ck = 0 if m == 0 else 3
                        nc.tensor.matmul(ps[:], wc[cb:cb + 64, m, :],
                                         img_t[cb:cb + 64, ck, :],
                                         start=False, stop=True)
                        tmp_m = tmp_pool.tile([P, W], F32, tag="tmp")
                        nc.scalar.activation(
                            tmp_m[:], ps[:],
                            mybir.ActivationFunctionType.Copy, scale=h2)

                        acc = out_t[:, ci, m, :]
                        nc.vector.scalar_tensor_tensor(
                            acc, _even(tmp_m[:]), r1, _odd(tmp_m[:]),
                            MULT, ADD)
                        nc.vector.scalar_tensor_tensor(
                            acc[:, 1:], _even(tmp_m[:, 1:511]), r0,
                            acc[:, 1:], MULT, ADD)
                        nc.vector.scalar_tensor_tensor(
                            acc[:, 0:255], _even(tmp_m[:, 2:512]), r3,
                            acc[:, 0:255], MULT, ADD)
                nc.scalar.dma_start(
                    out_d[i0:i0 + GB].rearrange("c (m p) q -> p (c m) q", p=P),
                    out_t[:].rearrange("p c m q -> p (c m) q"))

        if rounds == 1:
            body()
        else:
            with tc.For_i(0, rounds, 1):
                body()


def _emit_stencil16(nc, tc, x_d, mht_d, out_d, taps, rounds, out_dt=F32):
    """Both DMA mappings flipped to partition-contiguous: loads use the
    '(p k)' row layout (8 KB contiguous per partition), and the stage-1
    weight columns are permuted so PSUM tile u holds output rows 2p+u —
    every tile contracts over all 512 input rows (4 K=128 matmuls, no
    corner fixups or extra inputs), and stores write rows 2p, 2p+1 per
    partition (2 KB contiguous pieces)."""
    h0, h1, h2, h3 = (float(t) for t in taps)
    r0 = h0 / h2 if h2 else 0.0
    r1 = h1 / h2 if h2 else 0.0
    r3 = h3 / h2 if h2 else 0.0
    ib = int(os.environ.get("DWT_IB", "4"))
    pb = int(os.environ.get("DWT_PB", "4"))
    GB = int(os.environ.get("DWT_GB", "8"))
    mm_dt = F32R if os.environ.get("DWT_MM", "f32r") == "f32r" else F32
    with (
        tc.tile_pool(name="consts", bufs=1) as cpool,
        tc.tile_pool(name="img", bufs=ib) as img_pool,
        tc.tile_pool(name="tmp", bufs=ib) as tmp_pool,
        tc.tile_pool(name="outp", bufs=2) as out_pool,
        tc.tile_pool(name="ps1", bufs=pb, space=bass.MemorySpace.PSUM) as ps1_pool,
    ):
        # mht_t[p, k, q] = mht[4p+k, q]; the even/odd column split for
        # tile u is a stride-2 SBUF read at weight-load time.
        mht_t = cpool.tile([P, KH, NQ], mm_dt)
        nc.sync.dma_start(
            mht_t[:],
            mht_d.rearrange("(p k) q -> p k q", k=KH).bitcast(mm_dt))

        def body():
            for i0 in range(0, C, GB):
                out_t = out_pool.tile([P, GB, 2, NQ], out_dt, tag="out")
                for ci in range(GB):
                    i = i0 + ci
                    img_t = img_pool.tile([P, KH, W], mm_dt, tag="img")
                    nc.sync.dma_start(
                        img_t[:],
                        x_d[i].rearrange("(p k) w -> p k w", k=KH)
                        .bitcast(mm_dt))
                    for u in range(2):
                        ps = ps1_pool.tile([P, W], F32, tag="ps1")
                        for k in range(KH):
                            lhsT = mht_t[:, k, :].rearrange(
                                "p (q t) -> p t q", t=2)[:, u, :]
                            nc.tensor.matmul(ps[:], lhsT,
                                             img_t[:, k, :],
                                             start=(k == 0),
                                             stop=(k == KH - 1))
                        tmp_m = tmp_pool.tile([P, W], F32, tag="tmp")
                        nc.scalar.activation(
                            tmp_m[:], ps[:],
                            mybir.ActivationFunctionType.Copy, scale=h2)

                        acc = out_t[:, ci, u, :]
                        nc.vector.scalar_tensor_tensor(
                            acc, _even(tmp_m[:]), r1, _odd(tmp_m[:]),
                            MULT, ADD)
                        nc.vector.scalar_tensor_tensor(
                            acc[:, 1:], _even(tmp_m[:, 1:511]), r0,
                            acc[:, 1:], MULT, ADD)
                        nc.vector.scalar_tensor_tensor(
                            acc[:, 0:255], _even(tmp_m[:, 2:512]), r3,
                            acc[:, 0:255], MULT, ADD)
                nc.scalar.dma_start(
                    out_d[i0:i0 + GB].rearrange("c (p u) q -> p c u q", u=2),
                    out_t[:])

        if rounds == 1:
            body()
        else:
            with tc.For_i(0, rounds, 1):
                body()


def _emit_dmaonly(nc, tc, x_d, mht_d, out_d, taps, rounds, out_dt=F32):
    """DMA-streaming probe: moves the full per-round traffic (64 MB in,
    out stores) with no compute coupling. Output is all-zeros (timing
    only). Knobs: DWT_LB images per load, DWT_GB per store, DWT_ALT=1
    alternates loads across the SP and ACT HWDGE rings."""
    ib = int(os.environ.get("DWT_IB", "4"))
    GB = int(os.environ.get("DWT_GB", "8"))
    LB = int(os.environ.get("DWT_LB", "1"))
    ALT = os.environ.get("DWT_ALT", "0") == "1"
    with (
        tc.tile_pool(name="consts", bufs=1) as cpool,
        tc.tile_pool(name="img", bufs=ib) as img_pool,
    ):
        zo = cpool.tile([P, GB, 2, NQ], out_dt, name="zo")
        nc.gpsimd.memset(zo[:], 0.0)

        def body():
            for n, i0 in enumerate(range(0, C, LB)):
                t = img_pool.tile([P, LB, KH, W], F32, tag="img")
                eng = nc.scalar if (ALT and n % 2) else nc.sync
                eng.dma_start(
                    t[:],
                    x_d[i0:i0 + LB].rearrange("c (k p) w -> p c k w", p=P))
            for g0 in range(0, C, GB):
                nc.scalar.dma_start(
                    out_d[g0:g0 + GB].rearrange("c (m p) q -> p (c m) q", p=P),
                    zo[:].rearrange("p c m q -> p (c m) q"))

        if rounds == 1:
            body()
        else:
            with tc.For_i(0, rounds, 1):
                body()


def _emit_stencil7(nc, tc, x_d, mht_d, out_d, taps, rounds):
    """stencil6 + output DMA batched over groups of 4 images (1 MB per
    store) for better HBM store efficiency; input stays per-image so
    compute never waits on a multi-image load."""
    h0, h1, h2, h3 = (float(t) for t in taps)
    r0 = h0 / h2 if h2 else 0.0
    r1 = h1 / h2 if h2 else 0.0
    r3 = h3 / h2 if h2 else 0.0
    ib = int(os.environ.get("DWT_IB", "3"))
    pb = int(os.environ.get("DWT_PB", "4"))
    GB = 4  # images per output-DMA group
    with (
        tc.tile_pool(name="consts", bufs=1) as cpool,
        tc.tile_pool(name="img", bufs=ib) as img_pool,
        tc.tile_pool(name="tmp", bufs=ib) as tmp_pool,
        tc.tile_pool(name="outp", bufs=2) as out_pool,
        tc.tile_pool(name="corner", bufs=2) as corner_pool,
        tc.tile_pool(name="ps1", bufs=pb, space=bass.MemorySpace.PSUM) as ps1_pool,
    ):
        mht_t = cpool.tile([P, KH, NQ], F32)
        nc.sync.dma_start(mht_t[:], mht_d.rearrange("(k p) q -> p k q", p=P))

        def body():
            xc = corner_pool.tile([C, 2, W], F32, tag="xc")
            nc.sync.dma_start(xc[:], x_d[:, 255:257, :])
            sc = corner_pool.tile([C, 2, W], F32, tag="sc")
            nc.vector.tensor_scalar_mul(sc[:, 0], xc[:, 0], h2 * h0)
            nc.vector.tensor_scalar_mul(sc[:, 1], xc[:, 1], h2 * h3)

            for i0 in range(0, C, GB):
                out_t = out_pool.tile([P, GB, 2, NQ], F32, tag="out")
                for ci in range(GB):
                    i = i0 + ci
                    img_t = img_pool.tile([P, KH, W], F32, tag="img")
                    nc.sync.dma_start(
                        img_t[:], x_d[i].rearrange("(k p) w -> p k w", p=P))
                    for m in range(2):
                        ps = ps1_pool.tile([P, W], F32, tag="ps1")
                        for j in range(2):
                            k = 2 * m + j
                            lhsT = mht_t[:, k, m * P:(m + 1) * P]
                            nc.tensor.matmul(ps[:], lhsT, img_t[:, k, :],
                                             start=(j == 0), stop=(j == 1))
                        tmp_m = tmp_pool.tile([P, W], F32, tag="tmp")
                        nc.scalar.activation(
                            tmp_m[:], ps[:],
                            mybir.ActivationFunctionType.Copy, scale=h2)
                        if m == 0:
                            nc.gpsimd.dma_start(tmp_m[127:128, :],
                                                sc[i:i + 1, 1, :],
                                                accum_op=ADD)
                        else:
                            nc.gpsimd.dma_start(tmp_m[0:1, :],
                                                sc[i:i + 1, 0, :],
                                                accum_op=ADD)

                        acc = out_t[:, ci, m, :]
                        nc.vector.scalar_tensor_tensor(
                            acc, _even(tmp_m[:]), r1, _odd(tmp_m[:]),
                            MULT, ADD)
                        nc.vector.scalar_tensor_tensor(
                            acc[:, 1:], _even(tmp_m[:, 1:511]), r0,
                            acc[:, 1:], MULT, ADD)
                        nc.vector.scalar_tensor_tensor(
                            acc[:, 0:255], _even(tmp_m[:, 2:512]), r3,
                            acc[:, 0:255], MULT, ADD)
                nc.sync.dma_start(
                    out_d[i0:i0 + GB].rearrange("c (m p) q -> p (c m) q", p=P),
                    out_t[:].rearrange("p c m q -> p (c m) q"))

        if rounds == 1:
            body()
        else:
            with tc.For_i(0, rounds, 1):
                body()


def _emit_stencil8(nc, tc, x_d, mht_d, out_d, taps, rounds):
    """stencil6 with a restructured stage 2 that moves most DVE cycles to
    unit-stride (fp32 2x) ops: u[j] = r1*ts[j] + ts[j+1] and
    w[j] = (r3/r0)*ts[j+3] + ts[j] are unit-stride pair-combines; then one
    strided op acc[q] = r0*w[2q-1] + u[2q] plus two 1-column edge ops.
    Requires h0 != 0 (true for db2; degenerate all-zero matrices take the
    r0 == 0 fallback which is also exact)."""
    h0, h1, h2, h3 = (float(t) for t in taps)
    r0 = h0 / h2 if h2 else 0.0
    r1 = h1 / h2 if h2 else 0.0
    r3 = h3 / h2 if h2 else 0.0
    r30 = h3 / h0 if h0 else 0.0
    ib = int(os.environ.get("DWT_IB", "3"))
    pb = int(os.environ.get("DWT_PB", "4"))
    with (
        tc.tile_pool(name="consts", bufs=1) as cpool,
        tc.tile_pool(name="img", bufs=ib) as img_pool,
        tc.tile_pool(name="tmp", bufs=ib) as tmp_pool,
        tc.tile_pool(name="uw", bufs=ib) as uw_pool,
        tc.tile_pool(name="outp", bufs=ib) as out_pool,
        tc.tile_pool(name="corner", bufs=2) as corner_pool,
        tc.tile_pool(name="ps1", bufs=pb, space=bass.MemorySpace.PSUM) as ps1_pool,
    ):
        mht_t = cpool.tile([P, KH, NQ], F32)
        nc.sync.dma_start(mht_t[:], mht_d.rearrange("(k p) q -> p k q", p=P))

        def body():
            xc = corner_pool.tile([C, 2, W], F32, tag="xc")
            nc.sync.dma_start(xc[:], x_d[:, 255:257, :])
            sc = corner_pool.tile([C, 2, W], F32, tag="sc")
            nc.vector.tensor_scalar_mul(sc[:, 0], xc[:, 0], h2 * h0)
            nc.vector.tensor_scalar_mul(sc[:, 1], xc[:, 1], h2 * h3)

            for i in range(C):
                img_t = img_pool.tile([P, KH, W], F32, tag="img")
                nc.sync.dma_start(img_t[:],
                                  x_d[i].rearrange("(k p) w -> p k w", p=P))

                out_t = out_pool.tile([P, 2, NQ], F32, tag="out")
                for m in range(2):
                    ps = ps1_pool.tile([P, W], F32, tag="ps1")
                    for j in range(2):
                        k = 2 * m + j
                        lhsT = mht_t[:, k, m * P:(m + 1) * P]
                        nc.tensor.matmul(ps[:], lhsT, img_t[:, k, :],
                                         start=(j == 0), stop=(j == 1))
                    ts_m = tmp_pool.tile([P, W], F32, tag="tmp")
                    nc.scalar.activation(
                        ts_m[:], ps[:],
                        mybir.ActivationFunctionType.Copy, scale=h2)
                    if m == 0:
                        nc.gpsimd.dma_start(ts_m[127:128, :],
                                            sc[i:i + 1, 1, :], accum_op=ADD)
                    else:
                        nc.gpsimd.dma_start(ts_m[0:1, :],
                                            sc[i:i + 1, 0, :], accum_op=ADD)

                    acc = out_t[:, m, :]
                    if h0 == 0.0:
                        # degenerate fallback (exact for all-zero matrices)
                        nc.vector.scalar_tensor_tensor(
                            acc, _even(ts_m[:]), r1, _odd(ts_m[:]), MULT, ADD)
                        nc.vector.scalar_tensor_tensor(
                            acc[:, 0:255], _even(ts_m[:, 2:512]), r3,
                            acc[:, 0:255], MULT, ADD)
                        continue
                    u = uw_pool.tile([P, W], F32, tag="u")
                    w = uw_pool.tile([P, W], F32, tag="w")
                    # unit-stride pair-combines (fp32 2x mode)
                    nc.vector.scalar_tensor_tensor(
                        u[:, 0:511], ts_m[:, 0:511], r1, ts_m[:, 1:512],
                        MULT, ADD)
                    nc.vector.scalar_tensor_tensor(
                        w[:, 0:509], ts_m[:, 3:512], r30, ts_m[:, 0:509],
                        MULT, ADD)
                    # interior: acc[q] = r0*w[2q-1] + u[2q], q = 1..254
                    nc.vector.scalar_tensor_tensor(
                        acc[:, 1:255],
                        _odd(w[:, 0:508]), r0, _even(u[:, 2:510]),
                        MULT, ADD)
                    # edges: q=0 (h0 clipped), q=255 (h3 clipped)
                    nc.vector.scalar_tensor_tensor(
                        acc[:, 0:1], ts_m[:, 2:3], r3, u[:, 0:1], MULT, ADD)
                    nc.vector.scalar_tensor_tensor(
                        acc[:, 255:256], ts_m[:, 509:510], r0,
                        u[:, 510:511], MULT, ADD)
                nc.sync.dma_start(out_d[i].rearrange("(m p) q -> p m q", p=P),
                                  out_t[:])

        if rounds == 1:
            body()
        else:
            with tc.For_i(0, rounds, 1):
                body()


def _emit_stencil4(nc, tc, x_d, mht_d, out_d, taps, rounds):
    """stencil3 + input/output DMA batched over pairs of images (2 MB in /
    512 KB out per dma_start) for higher HBM DMA efficiency."""
    h0, h1, h2, h3 = (float(t) for t in taps)
    r0 = h0 / h2 if h2 else 0.0
    r1 = h1 / h2 if h2 else 0.0
    r3 = h3 / h2 if h2 else 0.0
    with (
        tc.tile_pool(name="consts", bufs=1) as cpool,
        tc.tile_pool(name="img", bufs=3) as img_pool,
        tc.tile_pool(name="tmp", bufs=3) as tmp_pool,
        tc.tile_pool(name="outp", bufs=3) as out_pool,
        tc.tile_pool(name="ps1", bufs=4, space=bass.MemorySpace.PSUM) as ps1_pool,
    ):
        mht_t = cpool.tile([P, KH, NQ], F32)
        nc.sync.dma_start(mht_t[:], mht_d.rearrange("(k p) q -> p k q", p=P))

        zb = [cpool.tile([P, W], F32, tag=f"zb{z}", name=f"zb{z}")
              for z in range(2)]
        for z in range(2):
            nc.gpsimd.memset(zb[z][:], 0.0)

        def image_pair(i0):
            img_t = img_pool.tile([P, 2, KH, W], F32, tag="img")
            nc.sync.dma_start(
                img_t[:],
                x_d[i0:i0 + 2].rearrange("c (k p) w -> p c k w", p=P))
            # corner rows for both images: x[255] and x[256]
            xr = img_pool.tile([1, 2, 2, W], F32, tag="xr")
            nc.sync.dma_start(
                xr[:], x_d[i0:i0 + 2, 255:257, :].unsqueeze(0))
            out_t = out_pool.tile([P, 2, 2, NQ], F32, tag="out")
            for ci in range(2):
                i = i0 + ci
                zb_i = zb[ci]
                nc.sync.dma_start(zb_i[127:128, :], x_d[i, 256:257, :])
                for m in range(2):
                    ps = ps1_pool.tile([P, W], F32, tag="ps1")
                    for j in range(2):
                        k = 2 * m + j
                        lhsT = mht_t[:, k, m * P:(m + 1) * P]
                        nc.tensor.matmul(ps[:], lhsT, img_t[:, ci, k, :],
                                         start=(j == 0), stop=(j == 1))
                    tmp_m = tmp_pool.tile([P, W], F32, tag="tmp")
                    nc.scalar.activation(
                        tmp_m[:], ps[:],
                        mybir.ActivationFunctionType.Copy, scale=h2)
                    if m == 0:
                        # tmp_s[127] += h2*h3 * x[256]
                        nc.vector.scalar_tensor_tensor(
                            tmp_m[:], zb_i[:], h2 * h3, tmp_m[:], MULT, ADD)
                    else:
                        # tmp_s[128] += h2*h0 * x[255]
                        nc.vector.scalar_tensor_tensor(
                            tmp_m[0:1, :], xr[:, ci, 0, :], h2 * h0,
                            tmp_m[0:1, :], MULT, ADD)

                    acc = out_t[:, ci, m, :]
                    nc.vector.scalar_tensor_tensor(
                        acc, _even(tmp_m[:]), r1, _odd(tmp_m[:]), MULT, ADD)
                    nc.vector.scalar_tensor_tensor(
                        acc[:, 1:], _even(tmp_m[:, 1:511]), r0,
                        acc[:, 1:], MULT, ADD)
                    nc.vector.scalar_tensor_tensor(
                        acc[:, 0:255], _even(tmp_m[:, 2:512]), r3,
                        acc[:, 0:255], MULT, ADD)
            nc.sync.dma_start(
                out_d[i0:i0 + 2].rearrange("c (m p) q -> p c m q", p=P),
                out_t[:])

        def body():
            for i0 in range(0, C, 2):
                image_pair(i0)

        if rounds == 1:
            body()
        else:
            with tc.For_i(0, rounds, 1):
                body()


def _emit_stencil2(nc, tc, x_d, mht_d, out_d, taps, rounds):
    """Like stencil, but the matmul rhs stream is column-permuted (all even
    w's, then all odd w's) so tmp lands deinterleaved in PSUM and every
    stage-2 stencil operand is unit-stride (DVE 2x fp32 mode)."""
